# revision 21
# baseline (speedup 1.0000x reference)
"""Multi-head causal attention (B=2, S=2048, D=1024, H=16, hd=64) on 8 trn2 cores.

Sharding: core c handles batch b = c//4 and head-group g = c%4 (heads 4g..4g+4,
d-slice 256g..256g+256 of the QKV projections / Wo rows).  Each core computes a
partial out-projection [2048, 1024] in bf16; the host sums the 4 head-group
partials per batch in f32 and adds the bias.

Per-core kernel (all matmuls bf16, accumulate f32 in PSUM):
  qT/kT = (x @ Wq/k)^T computed directly as [256, 2048] via lhsT=W chunks.
  v     = x @ Wv in natural [seq, head, 66] layout (col 64 = 1.0 so the
          attention rowsum falls out of the ctx matmul; col 65 = 0 pad).
  S^T   = k_h @ q_h^T  [kpos, qpos] tiles, both heads of a pair concurrently
          via PE row tiling; causal masking by accumulating maskU^T @ I =
          -1e30 (strict lower local triangle) into the score PSUM with one
          extra matmul on diagonal blocks, then exp via ACT (scale=1/8).
  ctx~T = v'_h^T @ expS^T accumulated over kpos blocks -> [66, 512] PSUM
          (row 64 = softmax denominator).
  out  += (ctx~T / rowsum)^T @ Wo rows.  Normalization is all on-chip: the
          two heads' rowsum rows are partition-broadcast with two col-tiled
          K=1 fp32 matmuls into one PSUM tile, copied to SBUF, reciprocal'd
          once [128, 512], and multiplied into ctxT.

Scheduling: attention groups run ascending, pairs alternating
((p0,g0),(p1,g0),(p0,g1),...) so exp starts early.  Independent PE work
(v/qk projections for later groups, out-projections for earlier ones) is
interleaved between attention steps by a debt-based filler scheduler so the
PE engine queue never stalls on the ACT exp pipeline (keeps HAM at K=8/8).
DMA issue cost (~650ns per dma_start on the issuing engine queue) is spread
across the sync/vector/scalar/gpsimd queues, with one batched DMA per weight
tensor.  A dozen warmup matmuls (made observable via a scrap DRAM write so
DCE keeps them) pre-warm the PE clock during the initial DMA fill.
"""

import sys

import numpy as np

for _p in ("/opt/trn_rl_repo",):
    if _p not in sys.path:
        sys.path.insert(0, _p)

import ml_dtypes

import concourse.bass as bass
import concourse.mybir as mybir
import concourse.tile as tile
from concourse import bacc
from concourse.bass_utils import run_bass_kernel_spmd
from concourse.masks import make_identity, make_upper_triangular

BF16 = mybir.dt.bfloat16
F32 = mybir.dt.float32

B, S, D, H, HD = 2, 2048, 1024, 16, 64
NCORES = 8
HPC = 4          # heads per core
DHC = HPC * HD   # 256: d-slice per core
P = 128
SB = S // P      # 16 seq blocks
KC = D // P      # 8 contraction chunks for projections
QG = 512         # q column group width
NQG = S // QG    # 4
VW = HD + 2      # 66: v cols per head (64 data + ones + pad; even M for PE)

# attention group order: (pair, g) ascending g, pairs alternating
ATT_SEQ = [(0, 0), (1, 0), (0, 1), (1, 1), (0, 2), (1, 2), (0, 3), (1, 3)]


def _build_body(ctx, tc, io):
    nc = tc.nc
    xT, wq, wk, wv, wo, out = (
        io["xT"], io["wq"], io["wk"], io["wv"], io["wo"], io["out"],
    )

    consts = ctx.enter_context(tc.tile_pool(name="consts", bufs=1))
    persist = ctx.enter_context(tc.tile_pool(name="persist", bufs=1))
    psum = ctx.enter_context(tc.tile_pool(name="psum", bufs=2, space="PSUM"))
    espool = ctx.enter_context(tc.tile_pool(name="es", bufs=6))
    nrmpool = ctx.enter_context(tc.tile_pool(name="nrm", bufs=2))
    outpool = ctx.enter_context(tc.tile_pool(name="outsb", bufs=3))
    drampool = ctx.enter_context(tc.tile_pool(name="dram", bufs=1, space="DRAM"))

    # maskU[a, b] = -1e30 iff a < b (strict upper).  Used as lhsT in the
    # causal-mask matmul: (maskU^T @ I)[i, j] = maskU[j, i] = -1e30 iff j < i.
    maskU = consts.tile([P, P], BF16, tag="masku", name="masku")
    make_upper_triangular(nc, maskU[:], val=-1e30, diag=False)
    ident = consts.tile([P, P], BF16, tag="ident", name="ident")
    make_identity(nc, ident[:])
    ones = consts.tile([1, HD], BF16, tag="ones", name="ones")
    nc.vector.memset(ones[:], 1.0)
    # warmup operand for PE clock pre-warm (values irrelevant)
    wup = consts.tile([P, QG], BF16, tag="wup", name="wup")
    nc.vector.memset(wup[:], 0.0)

    # ---- input DMAs.  Weights are host-packed partition-contiguous
    # ([128, ...] layouts) so lines are 1-2KB; everything is split into
    # ~128KB pieces so up to 16 DMA engines run in parallel (~22.5 GB/s
    # per engine), and issue cost (~650ns per dma_start) is spread across
    # the sync/scalar/gpsimd queues in strict priority order ----
    xt = [persist.tile([P, S], BF16, tag=f"xt{k}", name=f"xt{k}")
          for k in range(KC)]
    wv_sb = persist.tile([P, KC, DHC], BF16, tag="wv", name="wv_sb")
    wq_sb = persist.tile([P, KC, DHC], BF16, tag="wq", name="wq_sb")
    wk_sb = persist.tile([P, KC, DHC], BF16, tag="wk", name="wk_sb")
    wo_sb = persist.tile([P, 2, D], BF16, tag="wo", name="wo_sb")

    qs = [nc.sync, nc.scalar, nc.gpsimd]
    qi = [0]

    def dma(out_ap, in_ap):
        qs[qi[0] % 3].dma_start(out=out_ap, in_=in_ap)
        qi[0] += 1

    wv_f = wv_sb[:].rearrange("p k c -> p (k c)")
    wq_f = wq_sb[:].rearrange("p k c -> p (k c)")
    wk_f = wk_sb[:].rearrange("p k c -> p (k c)")
    wo_f = wo_sb[:].rearrange("p k c -> p (k c)")
    # priority set: wv + x j0 + wq + wk (interleaved in consumption order)
    for h in range(4):
        dma(xt[2 * h][:, 0:QG], xT[2 * h * P:(2 * h + 1) * P, 0:QG])
        dma(wv_f[:, h * QG:(h + 1) * QG], wv[:, h * QG:(h + 1) * QG])
        dma(xt[2 * h + 1][:, 0:QG],
            xT[(2 * h + 1) * P:(2 * h + 2) * P, 0:QG])
    for h in range(4):
        dma(wq_f[:, h * QG:(h + 1) * QG], wq[:, h * QG:(h + 1) * QG])
    for h in range(4):
        dma(wk_f[:, h * QG:(h + 1) * QG], wk[:, h * QG:(h + 1) * QG])
    # second wave: x j1, wo
    for k in range(KC):
        dma(xt[k][:, QG:2 * QG], xT[k * P:(k + 1) * P, QG:2 * QG])
    for h in range(4):
        dma(wo_f[:, h * QG:(h + 1) * QG], wo[:, h * QG:(h + 1) * QG])
    # third wave: x j2, j3
    for k in range(KC):
        dma(xt[k][:, 2 * QG:4 * QG], xT[k * P:(k + 1) * P, 2 * QG:4 * QG])

    # ---- PE clock warmup: one accumulation chain so every matmul feeds the
    # observable scrap DRAM write (else DCE drops them) ----
    scrap_d = drampool.tile([1, 16], F32, tag="scrap", name="scrap_d")
    wps = psum.tile([P, QG], F32, tag="pj", name="wps")
    for i in range(18):
        nc.tensor.matmul(wps[:], lhsT=wup[:, 0:P], rhs=wup[:],
                         start=(i == 0), stop=(i == 17))
    scrap_s = consts.tile([1, 16], F32, tag="scrap", name="scrap_s")
    nc.vector.tensor_copy(scrap_s[:], wps[0:1, 0:16])
    nc.gpsimd.dma_start(out=scrap_d[:], in_=scrap_s[:])

    # persistent tensors
    v_sb = [persist.tile([P, HPC, VW], BF16, tag=f"v{s}", name=f"v{s}")
            for s in range(SB)]
    qt = [persist.tile([P, S], BF16, tag=f"qt{i}", name=f"qt{i}")
          for i in range(2)]
    kt = [persist.tile([P, S], BF16, tag=f"kt{i}", name=f"kt{i}")
          for i in range(2)]
    ctxT = [persist.tile([P, S], BF16, tag=f"ctxT{i}", name=f"ctxT{i}")
            for i in range(2)]

    # ---- emission helpers ----
    def emit_v_chunk(sv):
        # two seq blocks (2*sv, 2*sv+1) -> v natural layout
        ps = psum.tile([P, 2, QG], F32, tag="sp", name="vps")
        for par in range(2):
            s = 2 * sv + par
            for k in range(KC):
                nc.tensor.matmul(
                    ps[:, par, 0:DHC],
                    lhsT=xt[k][:, s * P:(s + 1) * P],
                    rhs=wv_sb[:, k, :],
                    start=(k == 0),
                    stop=(k == KC - 1),
                )
            src_ap = ps[:, par, 0:DHC].rearrange("p (h d) -> p h d", h=HPC)
            nc.vector.tensor_copy(v_sb[s][:, :, 0:HD], src_ap)
            nc.vector.memset(v_sb[s][:, :, HD:VW], 1.0)
            nc.vector.memset(v_sb[s][:, :, HD + 1:VW], 0.0)

    def emit_qk_quantum(pair, which, j):
        # one projection accumulation: q or k, d-chunk `pair`, column group j
        w_sb, dst = ((wq_sb, qt) if which == "q" else (wk_sb, kt))
        ps = psum.tile([P, QG], F32, tag="pj", name="pjps")
        for k in range(KC):
            nc.tensor.matmul(
                ps[:],
                lhsT=w_sb[:, k, pair * P:(pair + 1) * P],
                rhs=xt[k][:, j * QG:(j + 1) * QG],
                start=(k == 0),
                stop=(k == KC - 1),
            )
        nc.vector.tensor_copy(dst[pair][:, j * QG:(j + 1) * QG], ps[:])

    out_tiles = {}

    def emit_outproj_half(m, n2, copy_eng=None):
        # out[m*128:(m+1)*128, n2*512:(n2+1)*512] partial projection
        if n2 == 0:
            out_tiles[m] = outpool.tile([P, D], BF16, tag="ot", name="ot")
        ot = out_tiles[m]
        ps = psum.tile([P, QG], F32, tag="pj", name="ops")
        for kc in range(2):
            nc.tensor.matmul(
                ps[:],
                lhsT=ctxT[kc][:, m * P:(m + 1) * P],
                rhs=wo_sb[:, kc, n2 * QG:(n2 + 1) * QG],
                start=(kc == 0),
                stop=(kc == 1),
            )
        if copy_eng == "scalar":
            nc.scalar.copy(ot[:, n2 * QG:(n2 + 1) * QG], ps[:])
        else:
            nc.vector.tensor_copy(ot[:, n2 * QG:(n2 + 1) * QG], ps[:])
        if n2 == 1:
            nc.sync.dma_start(out=out[m * P:(m + 1) * P, :], in_=ot[:])
            del out_tiles[m]

    # ---- filler scheduler: (cost_us, deadline_idx, fn, ready_idx) ----
    fillers = []
    fillers += [(1.7, 1, lambda w=w: emit_qk_quantum(1, w, 0), 0)
                for w in "qk"]
    for g in range(1, 4):
        d0 = 2 * g
        fillers += [(1.7, d0, lambda sv=sv: emit_v_chunk(sv), 0)
                    for sv in (2 * g, 2 * g + 1)]
        fillers += [(1.7, d0, lambda w=w, g=g: emit_qk_quantum(0, w, g), 0)
                    for w in "qk"]
        fillers += [(1.7, min(d0 + 1, 6), lambda w=w, g=g: emit_qk_quantum(1, w, g), 0)
                    for w in "qk"]
    # out-projections for groups 0..1 trail into the ACT-bound later stages;
    # group g's out-proj may only be emitted after norm(1, g) -> ready_idx.
    # Groups 2..3 are reserved for the epilogue (PE filler during the final
    # normalization chain).
    for g in range(2):
        for m in range(4 * g, 4 * g + 4):
            for n2 in range(2):
                fillers.append(
                    (0.5, 99, lambda m=m, n2=n2: emit_outproj_half(m, n2),
                     2 * g + 2))

    state = {"debt": 0.0, "idx": 0}

    def pop_ready(force_deadline=None):
        while fillers:
            c, d, f, r = fillers[0]
            if r > state["idx"]:
                return
            if force_deadline is not None and d <= force_deadline:
                pass  # forced
            elif c > state["debt"]:
                return
            fillers.pop(0)
            f()
            state["debt"] = max(0.0, state["debt"] - c)
            if force_deadline is not None and (not fillers
                                               or fillers[0][1] > force_deadline):
                force_deadline = None

    def emit_attention(pair, g):
        cxs = [psum.tile([VW, QG], F32, tag="cx", name="cx") for _ in range(2)]
        nkb = 4 * g + 4
        es_ts = [None] * nkb
        c0s = [P * (kb - 4 * g) if kb >= 4 * g else 0 for kb in range(nkb)]

        def emit_ctx(kb):
            c0 = c0s[kb]
            for hh in range(2):
                h = 2 * pair + hh
                nc.tensor.matmul(
                    cxs[hh][:, c0:QG],
                    lhsT=v_sb[kb][:, h, :],
                    rhs=es_ts[kb][:, hh, c0:QG],
                    start=(kb == 0),
                    stop=(kb == nkb - 1),
                )
            es_ts[kb] = None

        for kb in range(nkb):
            c0 = c0s[kb]
            w = QG - c0
            sp_t = psum.tile([P, 2, QG], F32, tag="sp", name="sp")
            diag = kb >= 4 * g
            for hh in range(2):
                nc.tensor.matmul(
                    sp_t[:, hh, c0:QG],
                    lhsT=kt[pair][hh * HD:(hh + 1) * HD, kb * P:(kb + 1) * P],
                    rhs=qt[pair][hh * HD:(hh + 1) * HD,
                                 g * QG + c0:(g + 1) * QG],
                    start=True,
                    stop=not diag,
                    skip_group_check=True,
                )
            if diag:
                # accumulate -1e30 on the strict lower local triangle of the
                # diagonal 128-col region, for both heads at once (rhs = I
                # broadcast along the hh axis)
                i_ap = ident[:]
                i_b = bass.AP(i_ap.tensor, i_ap.offset,
                              [i_ap.ap[0], [0, 2], i_ap.ap[1]])
                nc.tensor.matmul(
                    sp_t[:, :, c0:c0 + P],
                    lhsT=maskU[:],
                    rhs=i_b,
                    start=False,
                    stop=True,
                    skip_group_check=True,
                )
            es_t = espool.tile([P, 2, QG], BF16, tag="es", name="es")
            nc.scalar.activation(
                es_t[:, :, c0:QG], sp_t[:, :, c0:QG],
                mybir.ActivationFunctionType.Exp, scale=0.125,
            )
            es_ts[kb] = es_t
            # pipeline: emit ctx for the previous block, fillers in between
            state["debt"] += max(0.0, (2 * w + 420) / 1200.0
                                 - (3 * w / 2400.0 + 0.06))
            if kb >= 1:
                pop_ready()
                emit_ctx(kb - 1)
        pop_ready()
        emit_ctx(nkb - 1)
        return cxs

    def emit_norm(pair, g, cxs):
        # broadcast both heads' rowsum rows (PSUM partition 64) to a full
        # [128, 512] tile via two col-tiled K=1 bf16 matmuls, then one
        # reciprocal and two multiplies.  No DMA involved.
        rbp = psum.tile([P, QG], F32, tag="pj", name="rbp")
        for hh in range(2):
            rs = nrmpool.tile([1, QG], BF16, tag=f"rs{hh}", name="rs")
            nc.vector.tensor_copy(rs[:], cxs[hh][HD:HD + 1, :])
            nc.tensor.matmul(
                rbp[hh * HD:(hh + 1) * HD, :],
                lhsT=ones[:],
                rhs=rs[:],
                start=True,
                stop=True,
                tile_position=(0, hh * HD),
            )
        rbi = nrmpool.tile([P, QG], F32, tag="rbi", name="rbi")
        nc.vector.reciprocal_approx_fast(rbi[:], rbp[:])
        for hh in range(2):
            nc.vector.tensor_mul(
                ctxT[pair][hh * HD:(hh + 1) * HD, g * QG:(g + 1) * QG],
                cxs[hh][0:HD, :],
                rbi[hh * HD:(hh + 1) * HD, :],
            )

    # ---- emission schedule ----
    emit_qk_quantum(0, "q", 0)
    emit_qk_quantum(0, "k", 0)
    emit_v_chunk(0)
    emit_v_chunk(1)
    for idx, (pair, g) in enumerate(ATT_SEQ):
        state["idx"] = idx
        pop_ready(force_deadline=idx)
        cxs = emit_attention(pair, g)
        emit_norm(pair, g, cxs)
    # epilogue: drain remaining fillers, then the reserved out-projections.
    # outproj(2) is independent of the final norm (cols 1024:1536, normed at
    # idx 5) so its PE work covers the norm's DVE chain; PSUM->SBUF copies
    # alternate between ScalarE (idle after the last exp) and VectorE so the
    # drain is PE-bound, which also keeps HAM warm.
    state["idx"] = 99
    state["debt"] = 1e9
    pop_ready(force_deadline=99)
    for i, m in enumerate(range(8, 16)):
        for n2 in range(2):
            emit_outproj_half(m, n2,
                              copy_eng=("scalar" if (2 * i + n2) % 2 else None))


def build_nc():
    from contextlib import ExitStack

    nc = bacc.Bacc()
    # weights are host-packed partition-contiguous: w*[p, k*C+c] = W[k*128+p, c]
    io = {
        "xT": nc.dram_tensor("xT", [D, S], BF16, kind="ExternalInput").ap(),
        "wq": nc.dram_tensor("wq", [P, KC * DHC], BF16, kind="ExternalInput").ap(),
        "wk": nc.dram_tensor("wk", [P, KC * DHC], BF16, kind="ExternalInput").ap(),
        "wv": nc.dram_tensor("wv", [P, KC * DHC], BF16, kind="ExternalInput").ap(),
        "wo": nc.dram_tensor("wo", [P, 2 * D], BF16, kind="ExternalInput").ap(),
        "out": nc.dram_tensor("out", [S, D], BF16, kind="ExternalOutput").ap(),
    }
    with tile.TileContext(nc) as tc:
        with ExitStack() as ctx:
            _build_body(ctx, tc, io)
    nc.finalize()
    return nc


_NC = None


def _get_nc():
    global _NC
    if _NC is None:
        _NC = build_nc()
    return _NC


def _pack(w, nchunk):
    # [nchunk*128, C] -> [128, nchunk*C] with w_packed[p, k*C+c] = w[k*128+p, c]
    bf = ml_dtypes.bfloat16
    w = np.asarray(w, np.float32)
    C = w.shape[1]
    return np.ascontiguousarray(
        w.reshape(nchunk, P, C).transpose(1, 0, 2).reshape(P, nchunk * C)
    ).astype(bf)


def make_in_maps(x, Wq, Wk, Wv, Wo):
    bf = ml_dtypes.bfloat16
    x = np.asarray(x, dtype=np.float32)
    Wq, Wk, Wv = (np.asarray(w, np.float32) for w in (Wq, Wk, Wv))
    Wo = np.asarray(Wo, np.float32)
    in_maps = []
    xTs = [np.ascontiguousarray(x[b].T).astype(bf) for b in range(B)]
    for c in range(NCORES):
        b, g = divmod(c, 4)
        sl = slice(DHC * g, DHC * (g + 1))
        in_maps.append({
            "xT": xTs[b],
            "wq": _pack(Wq[:, sl], KC),
            "wk": _pack(Wk[:, sl], KC),
            "wv": _pack(Wv[:, sl], KC),
            "wo": _pack(Wo[sl, :], 2),
        })
    return in_maps


def run(in_maps, trace=False, **kw):
    return run_bass_kernel_spmd(_get_nc(), in_maps, list(range(NCORES)),
                                trace=trace, **kw)


def kernel(x, Wq, Wk, Wv, Wo, bo):
    res = run(make_in_maps(x, Wq, Wk, Wv, Wo)).results
    bo = np.asarray(bo, np.float32)
    out = np.empty((B, S, D), np.float32)
    for b in range(B):
        acc = res[4 * b]["out"].astype(np.float32)
        for g in range(1, 4):
            acc = acc + res[4 * b + g]["out"].astype(np.float32)
        out[b] = acc + bo[None, :]
    return out


# revision 22
# speedup vs baseline: 1.0326x; 1.0326x over previous
"""Multi-head causal attention (B=2, S=2048, D=1024, H=16, hd=64) on 8 trn2 cores.

Sharding: core c handles batch b = c//4 and head-group g = c%4 (heads 4g..4g+4,
d-slice 256g..256g+256 of the QKV projections / Wo rows).  Each core computes a
partial out-projection [2048, 1024] in bf16; the host sums the 4 head-group
partials per batch in f32 and adds the bias.

Per-core kernel (all matmuls bf16, accumulate f32 in PSUM):
  qT/kT = (x @ Wq/k)^T computed directly as [256, 2048] via lhsT=W chunks.
  v     = x @ Wv in natural [seq, head, 66] layout (col 64 = 1.0 so the
          attention rowsum falls out of the ctx matmul; col 65 = 0 pad).
  S^T   = k_h @ q_h^T  [kpos, qpos] tiles, both heads of a pair concurrently
          via PE row tiling; causal masking by accumulating maskU^T @ I =
          -1e30 (strict lower local triangle) into the score PSUM with one
          extra matmul on diagonal blocks, then exp via ACT (scale=1/8).
  ctx~T = v'_h^T @ expS^T accumulated over kpos blocks -> [66, 512] PSUM
          (row 64 = softmax denominator).
  out  += (ctx~T / rowsum)^T @ Wo rows.  Normalization is all on-chip: the
          two heads' rowsum rows are partition-broadcast with two col-tiled
          K=1 fp32 matmuls into one PSUM tile, copied to SBUF, reciprocal'd
          once [128, 512], and multiplied into ctxT.

Scheduling: attention groups run ascending, pairs alternating
((p0,g0),(p1,g0),(p0,g1),...) so exp starts early.  Independent PE work
(v/qk projections for later groups, out-projections for earlier ones) is
interleaved between attention steps by a debt-based filler scheduler so the
PE engine queue never stalls on the ACT exp pipeline (keeps HAM at K=8/8).
DMA issue cost (~650ns per dma_start on the issuing engine queue) is spread
across the sync/vector/scalar/gpsimd queues, with one batched DMA per weight
tensor.  A dozen warmup matmuls (made observable via a scrap DRAM write so
DCE keeps them) pre-warm the PE clock during the initial DMA fill.
"""

import sys

import numpy as np

for _p in ("/opt/trn_rl_repo",):
    if _p not in sys.path:
        sys.path.insert(0, _p)

import ml_dtypes

import concourse.bass as bass
import concourse.mybir as mybir
import concourse.tile as tile
from concourse import bacc
from concourse.bass_utils import run_bass_kernel_spmd
from concourse.masks import make_identity, make_upper_triangular

BF16 = mybir.dt.bfloat16
F32 = mybir.dt.float32

B, S, D, H, HD = 2, 2048, 1024, 16, 64
NCORES = 8
HPC = 4          # heads per core
DHC = HPC * HD   # 256: d-slice per core
P = 128
SB = S // P      # 16 seq blocks
KC = D // P      # 8 contraction chunks for projections
QG = 512         # q column group width
NQG = S // QG    # 4
VW = HD + 2      # 66: v cols per head (64 data + ones + pad; even M for PE)

# attention group order: (pair, g) ascending g, pairs alternating
ATT_SEQ = [(0, 0), (1, 0), (0, 1), (1, 1), (0, 2), (1, 2), (0, 3), (1, 3)]


def _build_body(ctx, tc, io):
    nc = tc.nc
    xT, wq, wk, wv, wo, out = (
        io["xT"], io["wq"], io["wk"], io["wv"], io["wo"], io["out"],
    )

    consts = ctx.enter_context(tc.tile_pool(name="consts", bufs=1))
    persist = ctx.enter_context(tc.tile_pool(name="persist", bufs=1))
    psum = ctx.enter_context(tc.tile_pool(name="psum", bufs=2, space="PSUM"))
    espool = ctx.enter_context(tc.tile_pool(name="es", bufs=6))
    nrmpool = ctx.enter_context(tc.tile_pool(name="nrm", bufs=2))
    outpool = ctx.enter_context(tc.tile_pool(name="outsb", bufs=3))
    drampool = ctx.enter_context(tc.tile_pool(name="dram", bufs=1, space="DRAM"))

    # maskU[a, b] = -1e30 iff a < b (strict upper).  Used as lhsT in the
    # causal-mask matmul: (maskU^T @ I)[i, j] = maskU[j, i] = -1e30 iff j < i.
    maskU = consts.tile([P, P], BF16, tag="masku", name="masku")
    make_upper_triangular(nc, maskU[:], val=-1e30, diag=False)
    ident = consts.tile([P, P], BF16, tag="ident", name="ident")
    make_identity(nc, ident[:])
    ones = consts.tile([1, HD], BF16, tag="ones", name="ones")
    nc.vector.memset(ones[:], 1.0)
    # warmup operand for PE clock pre-warm (values irrelevant)
    wup = consts.tile([P, QG], BF16, tag="wup", name="wup")
    nc.vector.memset(wup[:], 0.0)

    # ---- input DMAs.  Weights are host-packed partition-contiguous
    # ([128, ...] layouts) so lines are 1-2KB; everything is split into
    # ~128KB pieces so up to 16 DMA engines run in parallel (~22.5 GB/s
    # per engine), and issue cost (~650ns per dma_start) is spread across
    # the sync/scalar/gpsimd queues in strict priority order ----
    xt = [persist.tile([P, S], BF16, tag=f"xt{k}", name=f"xt{k}")
          for k in range(KC)]
    wv_sb = persist.tile([P, KC, DHC], BF16, tag="wv", name="wv_sb")
    wq_sb = persist.tile([P, KC, DHC], BF16, tag="wq", name="wq_sb")
    wk_sb = persist.tile([P, KC, DHC], BF16, tag="wk", name="wk_sb")
    wo_sb = persist.tile([P, 2, D], BF16, tag="wo", name="wo_sb")

    qs = [nc.sync, nc.scalar, nc.gpsimd]
    qi = [0]

    def dma(out_ap, in_ap):
        qs[qi[0] % 3].dma_start(out=out_ap, in_=in_ap)
        qi[0] += 1

    wv_f = wv_sb[:].rearrange("p k c -> p (k c)")
    wq_f = wq_sb[:].rearrange("p k c -> p (k c)")
    wk_f = wk_sb[:].rearrange("p k c -> p (k c)")
    wo_f = wo_sb[:].rearrange("p k c -> p (k c)")
    # priority set: wv + x j0 + wq + wk (interleaved in consumption order)
    for h in range(4):
        dma(xt[2 * h][:, 0:QG], xT[2 * h * P:(2 * h + 1) * P, 0:QG])
        dma(wv_f[:, h * QG:(h + 1) * QG], wv[:, h * QG:(h + 1) * QG])
        dma(xt[2 * h + 1][:, 0:QG],
            xT[(2 * h + 1) * P:(2 * h + 2) * P, 0:QG])
    for h in range(4):
        dma(wq_f[:, h * QG:(h + 1) * QG], wq[:, h * QG:(h + 1) * QG])
    for h in range(4):
        dma(wk_f[:, h * QG:(h + 1) * QG], wk[:, h * QG:(h + 1) * QG])
    # second wave: x j1, wo
    for k in range(KC):
        dma(xt[k][:, QG:2 * QG], xT[k * P:(k + 1) * P, QG:2 * QG])
    for h in range(4):
        dma(wo_f[:, h * QG:(h + 1) * QG], wo[:, h * QG:(h + 1) * QG])
    # third wave: x j2, j3
    for k in range(KC):
        dma(xt[k][:, 2 * QG:4 * QG], xT[k * P:(k + 1) * P, 2 * QG:4 * QG])

    # ---- PE clock warmup: one accumulation chain so every matmul feeds the
    # observable scrap DRAM write (else DCE drops them) ----
    scrap_d = drampool.tile([1, 16], F32, tag="scrap", name="scrap_d")
    wps = psum.tile([P, QG], F32, tag="pj", name="wps")
    for i in range(12):
        nc.tensor.matmul(wps[:], lhsT=wup[:, 0:P], rhs=wup[:],
                         start=(i == 0), stop=(i == 11))
    scrap_s = consts.tile([1, 16], F32, tag="scrap", name="scrap_s")
    nc.vector.tensor_copy(scrap_s[:], wps[0:1, 0:16])
    nc.gpsimd.dma_start(out=scrap_d[:], in_=scrap_s[:])

    # persistent tensors
    v_sb = [persist.tile([P, HPC, VW], BF16, tag=f"v{s}", name=f"v{s}")
            for s in range(SB)]
    qt = [persist.tile([P, S], BF16, tag=f"qt{i}", name=f"qt{i}")
          for i in range(2)]
    kt = [persist.tile([P, S], BF16, tag=f"kt{i}", name=f"kt{i}")
          for i in range(2)]
    ctxT = [persist.tile([P, S], BF16, tag=f"ctxT{i}", name=f"ctxT{i}")
            for i in range(2)]

    # ---- emission helpers ----
    def emit_v_chunk(sv):
        # two seq blocks (2*sv, 2*sv+1) -> v natural layout
        ps = psum.tile([P, 2, QG], F32, tag="sp", name="vps")
        for par in range(2):
            s = 2 * sv + par
            for k in range(KC):
                nc.tensor.matmul(
                    ps[:, par, 0:DHC],
                    lhsT=xt[k][:, s * P:(s + 1) * P],
                    rhs=wv_sb[:, k, :],
                    start=(k == 0),
                    stop=(k == KC - 1),
                )
            src_ap = ps[:, par, 0:DHC].rearrange("p (h d) -> p h d", h=HPC)
            nc.vector.tensor_copy(v_sb[s][:, :, 0:HD], src_ap)
            nc.vector.memset(v_sb[s][:, :, HD:VW], 1.0)
            nc.vector.memset(v_sb[s][:, :, HD + 1:VW], 0.0)

    def emit_qk_quantum(pair, which, j):
        # one projection accumulation: q or k, d-chunk `pair`, column group j
        w_sb, dst = ((wq_sb, qt) if which == "q" else (wk_sb, kt))
        ps = psum.tile([P, QG], F32, tag="pj", name="pjps")
        for k in range(KC):
            nc.tensor.matmul(
                ps[:],
                lhsT=w_sb[:, k, pair * P:(pair + 1) * P],
                rhs=xt[k][:, j * QG:(j + 1) * QG],
                start=(k == 0),
                stop=(k == KC - 1),
            )
        nc.vector.tensor_copy(dst[pair][:, j * QG:(j + 1) * QG], ps[:])

    out_tiles = {}

    def emit_outproj_half(m, n2, copy_eng=None):
        # out[m*128:(m+1)*128, n2*512:(n2+1)*512] partial projection
        if n2 == 0:
            out_tiles[m] = outpool.tile([P, D], BF16, tag="ot", name="ot")
        ot = out_tiles[m]
        ps = psum.tile([P, QG], F32, tag="pj", name="ops")
        for kc in range(2):
            nc.tensor.matmul(
                ps[:],
                lhsT=ctxT[kc][:, m * P:(m + 1) * P],
                rhs=wo_sb[:, kc, n2 * QG:(n2 + 1) * QG],
                start=(kc == 0),
                stop=(kc == 1),
            )
        if copy_eng == "scalar":
            nc.scalar.copy(ot[:, n2 * QG:(n2 + 1) * QG], ps[:])
        else:
            nc.vector.tensor_copy(ot[:, n2 * QG:(n2 + 1) * QG], ps[:])
        if n2 == 1:
            nc.sync.dma_start(out=out[m * P:(m + 1) * P, :], in_=ot[:])
            del out_tiles[m]

    # ---- filler scheduler: (cost_us, deadline_idx, fn, ready_idx) ----
    fillers = []
    fillers += [(1.7, 1, lambda w=w: emit_qk_quantum(1, w, 0), 0)
                for w in "qk"]
    for g in range(1, 4):
        d0 = 2 * g
        fillers += [(1.7, d0, lambda sv=sv: emit_v_chunk(sv), 0)
                    for sv in (2 * g, 2 * g + 1)]
        fillers += [(1.7, d0, lambda w=w, g=g: emit_qk_quantum(0, w, g), 0)
                    for w in "qk"]
        fillers += [(1.7, min(d0 + 1, 6), lambda w=w, g=g: emit_qk_quantum(1, w, g), 0)
                    for w in "qk"]
    # out-projections for groups 0..1 trail into the ACT-bound later stages;
    # group g's out-proj may only be emitted after norm(1, g) -> ready_idx.
    # Groups 2..3 are reserved for the epilogue (PE filler during the final
    # normalization chain).
    for g in range(2):
        for m in range(4 * g, 4 * g + 4):
            for n2 in range(2):
                fillers.append(
                    (0.5, 99, lambda m=m, n2=n2: emit_outproj_half(m, n2),
                     2 * g + 2))

    state = {"debt": 0.0, "idx": 0}

    def pop_ready(force_deadline=None):
        while fillers:
            c, d, f, r = fillers[0]
            if r > state["idx"]:
                return
            if force_deadline is not None and d <= force_deadline:
                pass  # forced
            elif c > state["debt"]:
                return
            fillers.pop(0)
            f()
            state["debt"] = max(0.0, state["debt"] - c)
            if force_deadline is not None and (not fillers
                                               or fillers[0][1] > force_deadline):
                force_deadline = None

    def emit_attention(pair, g):
        cxs = [psum.tile([VW, QG], F32, tag="cx", name="cx") for _ in range(2)]
        nkb = 4 * g + 4
        es_ts = [None] * nkb
        c0s = [P * (kb - 4 * g) if kb >= 4 * g else 0 for kb in range(nkb)]

        def emit_ctx(kb):
            c0 = c0s[kb]
            for hh in range(2):
                h = 2 * pair + hh
                nc.tensor.matmul(
                    cxs[hh][:, c0:QG],
                    lhsT=v_sb[kb][:, h, :],
                    rhs=es_ts[kb][:, hh, c0:QG],
                    start=(kb == 0),
                    stop=(kb == nkb - 1),
                )
            es_ts[kb] = None

        for kb in range(nkb):
            c0 = c0s[kb]
            w = QG - c0
            sp_t = psum.tile([P, 2, QG], F32, tag="sp", name="sp")
            diag = kb >= 4 * g
            for hh in range(2):
                nc.tensor.matmul(
                    sp_t[:, hh, c0:QG],
                    lhsT=kt[pair][hh * HD:(hh + 1) * HD, kb * P:(kb + 1) * P],
                    rhs=qt[pair][hh * HD:(hh + 1) * HD,
                                 g * QG + c0:(g + 1) * QG],
                    start=True,
                    stop=not diag,
                    skip_group_check=True,
                )
            if diag:
                # accumulate -1e30 on the strict lower local triangle of the
                # diagonal 128-col region, for both heads at once (rhs = I
                # broadcast along the hh axis)
                i_ap = ident[:]
                i_b = bass.AP(i_ap.tensor, i_ap.offset,
                              [i_ap.ap[0], [0, 2], i_ap.ap[1]])
                nc.tensor.matmul(
                    sp_t[:, :, c0:c0 + P],
                    lhsT=maskU[:],
                    rhs=i_b,
                    start=False,
                    stop=True,
                    skip_group_check=True,
                )
            es_t = espool.tile([P, 2, QG], BF16, tag="es", name="es")
            nc.scalar.activation(
                es_t[:, :, c0:QG], sp_t[:, :, c0:QG],
                mybir.ActivationFunctionType.Exp, scale=0.125,
            )
            es_ts[kb] = es_t
            # pipeline: emit ctx for the previous block, fillers in between
            state["debt"] += max(0.0, (2 * w + 420) / 1200.0
                                 - (3 * w / 2400.0 + 0.06))
            if kb >= 1:
                pop_ready()
                emit_ctx(kb - 1)
        pop_ready()
        emit_ctx(nkb - 1)
        return cxs

    def emit_norm(pair, g, cxs):
        # broadcast both heads' rowsum rows (PSUM partition 64) to a full
        # [128, 512] tile via two col-tiled K=1 bf16 matmuls, then one
        # reciprocal and two multiplies.  No DMA involved.
        rbp = psum.tile([P, QG], F32, tag="pj", name="rbp")
        for hh in range(2):
            rs = nrmpool.tile([1, QG], BF16, tag=f"rs{hh}", name="rs")
            nc.vector.tensor_copy(rs[:], cxs[hh][HD:HD + 1, :])
            nc.tensor.matmul(
                rbp[hh * HD:(hh + 1) * HD, :],
                lhsT=ones[:],
                rhs=rs[:],
                start=True,
                stop=True,
                tile_position=(0, hh * HD),
            )
        rbi = nrmpool.tile([P, QG], F32, tag="rbi", name="rbi")
        nc.vector.reciprocal_approx_fast(rbi[:], rbp[:])
        for hh in range(2):
            nc.vector.tensor_mul(
                ctxT[pair][hh * HD:(hh + 1) * HD, g * QG:(g + 1) * QG],
                cxs[hh][0:HD, :],
                rbi[hh * HD:(hh + 1) * HD, :],
            )

    # ---- emission schedule ----
    emit_v_chunk(0)
    emit_qk_quantum(0, "q", 0)
    emit_qk_quantum(0, "k", 0)
    emit_v_chunk(1)
    for idx, (pair, g) in enumerate(ATT_SEQ):
        state["idx"] = idx
        pop_ready(force_deadline=idx)
        cxs = emit_attention(pair, g)
        emit_norm(pair, g, cxs)
    # epilogue: drain remaining fillers, then the reserved out-projections.
    # outproj(2) is independent of the final norm (cols 1024:1536, normed at
    # idx 5) so its PE work covers the norm's DVE chain; PSUM->SBUF copies
    # alternate between ScalarE (idle after the last exp) and VectorE so the
    # drain is PE-bound, which also keeps HAM warm.
    state["idx"] = 99
    state["debt"] = 1e9
    pop_ready(force_deadline=99)
    for i, m in enumerate(range(8, 16)):
        for n2 in range(2):
            emit_outproj_half(m, n2,
                              copy_eng=("scalar" if (2 * i + n2) % 2 else None))


def build_nc():
    from contextlib import ExitStack

    nc = bacc.Bacc()
    # weights are host-packed partition-contiguous: w*[p, k*C+c] = W[k*128+p, c]
    io = {
        "xT": nc.dram_tensor("xT", [D, S], BF16, kind="ExternalInput").ap(),
        "wq": nc.dram_tensor("wq", [P, KC * DHC], BF16, kind="ExternalInput").ap(),
        "wk": nc.dram_tensor("wk", [P, KC * DHC], BF16, kind="ExternalInput").ap(),
        "wv": nc.dram_tensor("wv", [P, KC * DHC], BF16, kind="ExternalInput").ap(),
        "wo": nc.dram_tensor("wo", [P, 2 * D], BF16, kind="ExternalInput").ap(),
        "out": nc.dram_tensor("out", [S, D], BF16, kind="ExternalOutput").ap(),
    }
    with tile.TileContext(nc) as tc:
        with ExitStack() as ctx:
            _build_body(ctx, tc, io)
    nc.finalize()
    return nc


_NC = None


def _get_nc():
    global _NC
    if _NC is None:
        _NC = build_nc()
    return _NC


def _pack(w, nchunk):
    # [nchunk*128, C] -> [128, nchunk*C] with w_packed[p, k*C+c] = w[k*128+p, c]
    bf = ml_dtypes.bfloat16
    w = np.asarray(w, np.float32)
    C = w.shape[1]
    return np.ascontiguousarray(
        w.reshape(nchunk, P, C).transpose(1, 0, 2).reshape(P, nchunk * C)
    ).astype(bf)


def make_in_maps(x, Wq, Wk, Wv, Wo):
    bf = ml_dtypes.bfloat16
    x = np.asarray(x, dtype=np.float32)
    Wq, Wk, Wv = (np.asarray(w, np.float32) for w in (Wq, Wk, Wv))
    Wo = np.asarray(Wo, np.float32)
    in_maps = []
    xTs = [np.ascontiguousarray(x[b].T).astype(bf) for b in range(B)]
    for c in range(NCORES):
        b, g = divmod(c, 4)
        sl = slice(DHC * g, DHC * (g + 1))
        in_maps.append({
            "xT": xTs[b],
            "wq": _pack(Wq[:, sl], KC),
            "wk": _pack(Wk[:, sl], KC),
            "wv": _pack(Wv[:, sl], KC),
            "wo": _pack(Wo[sl, :], 2),
        })
    return in_maps


def run(in_maps, trace=False, **kw):
    return run_bass_kernel_spmd(_get_nc(), in_maps, list(range(NCORES)),
                                trace=trace, **kw)


def kernel(x, Wq, Wk, Wv, Wo, bo):
    res = run(make_in_maps(x, Wq, Wk, Wv, Wo)).results
    bo = np.asarray(bo, np.float32)
    out = np.empty((B, S, D), np.float32)
    for b in range(B):
        acc = res[4 * b]["out"].astype(np.float32)
        for g in range(1, 4):
            acc = acc + res[4 * b + g]["out"].astype(np.float32)
        out[b] = acc + bo[None, :]
    return out


# revision 23
# speedup vs baseline: 1.0400x; 1.0071x over previous
"""Multi-head causal attention (B=2, S=2048, D=1024, H=16, hd=64) on 8 trn2 cores.

Sharding: core c handles batch b = c//4 and head-group g = c%4 (heads 4g..4g+4,
d-slice 256g..256g+256 of the QKV projections / Wo rows).  Each core computes a
partial out-projection [2048, 1024] in bf16; the host sums the 4 head-group
partials per batch in f32 and adds the bias.

Per-core kernel (all matmuls bf16, accumulate f32 in PSUM):
  qT/kT = (x @ Wq/k)^T computed directly as [256, 2048] via lhsT=W chunks.
  v     = x @ Wv in natural [seq, head, 66] layout (col 64 = 1.0 so the
          attention rowsum falls out of the ctx matmul; col 65 = 0 pad).
  S^T   = k_h @ q_h^T  [kpos, qpos] tiles, both heads of a pair concurrently
          via PE row tiling; causal masking by accumulating maskU^T @ I =
          -1e30 (strict lower local triangle) into the score PSUM with one
          extra matmul on diagonal blocks, then exp via ACT (scale=1/8).
  ctx~T = v'_h^T @ expS^T accumulated over kpos blocks -> [66, 512] PSUM
          (row 64 = softmax denominator).
  out  += (ctx~T / rowsum)^T @ Wo rows.  Normalization is all on-chip: the
          two heads' rowsum rows are partition-broadcast with two col-tiled
          K=1 fp32 matmuls into one PSUM tile, copied to SBUF, reciprocal'd
          once [128, 512], and multiplied into ctxT.

Scheduling: attention groups run ascending, pairs alternating
((p0,g0),(p1,g0),(p0,g1),...) so exp starts early.  Independent PE work
(v/qk projections for later groups, out-projections for earlier ones) is
interleaved between attention steps by a debt-based filler scheduler so the
PE engine queue never stalls on the ACT exp pipeline (keeps HAM at K=8/8).
DMA issue cost (~650ns per dma_start on the issuing engine queue) is spread
across the sync/vector/scalar/gpsimd queues, with one batched DMA per weight
tensor.  A dozen warmup matmuls (made observable via a scrap DRAM write so
DCE keeps them) pre-warm the PE clock during the initial DMA fill.
"""

import sys

import numpy as np

for _p in ("/opt/trn_rl_repo",):
    if _p not in sys.path:
        sys.path.insert(0, _p)

import ml_dtypes

import concourse.bass as bass
import concourse.mybir as mybir
import concourse.tile as tile
from concourse import bacc
from concourse.bass_utils import run_bass_kernel_spmd
from concourse.masks import make_identity, make_upper_triangular

BF16 = mybir.dt.bfloat16
F32 = mybir.dt.float32

B, S, D, H, HD = 2, 2048, 1024, 16, 64
NCORES = 8
HPC = 4          # heads per core
DHC = HPC * HD   # 256: d-slice per core
P = 128
SB = S // P      # 16 seq blocks
KC = D // P      # 8 contraction chunks for projections
QG = 512         # q column group width
NQG = S // QG    # 4
VW = HD + 2      # 66: v cols per head (64 data + ones + pad; even M for PE)

# attention group order: (pair, g) ascending g, pairs alternating
ATT_SEQ = [(0, 0), (1, 0), (0, 1), (1, 1), (0, 2), (1, 2), (0, 3), (1, 3)]


def _build_body(ctx, tc, io):
    nc = tc.nc
    xT, wq, wk, wv, wo, out = (
        io["xT"], io["wq"], io["wk"], io["wv"], io["wo"], io["out"],
    )

    consts = ctx.enter_context(tc.tile_pool(name="consts", bufs=1))
    persist = ctx.enter_context(tc.tile_pool(name="persist", bufs=1))
    psum = ctx.enter_context(tc.tile_pool(name="psum", bufs=2, space="PSUM"))
    espool = ctx.enter_context(tc.tile_pool(name="es", bufs=6))
    nrmpool = ctx.enter_context(tc.tile_pool(name="nrm", bufs=2))
    outpool = ctx.enter_context(tc.tile_pool(name="outsb", bufs=5))
    drampool = ctx.enter_context(tc.tile_pool(name="dram", bufs=1, space="DRAM"))

    # maskU[a, b] = -1e30 iff a < b (strict upper).  Used as lhsT in the
    # causal-mask matmul: (maskU^T @ I)[i, j] = maskU[j, i] = -1e30 iff j < i.
    maskU = consts.tile([P, P], BF16, tag="masku", name="masku")
    make_upper_triangular(nc, maskU[:], val=-1e30, diag=False)
    ident = consts.tile([P, P], BF16, tag="ident", name="ident")
    make_identity(nc, ident[:])
    ones = consts.tile([1, HD], BF16, tag="ones", name="ones")
    nc.vector.memset(ones[:], 1.0)
    # warmup operand for PE clock pre-warm (values irrelevant)
    wup = consts.tile([P, QG], BF16, tag="wup", name="wup")
    nc.vector.memset(wup[:], 0.0)

    # ---- input DMAs.  Weights are host-packed partition-contiguous
    # ([128, ...] layouts) so lines are 1-2KB; everything is split into
    # ~128KB pieces so up to 16 DMA engines run in parallel (~22.5 GB/s
    # per engine), and issue cost (~650ns per dma_start) is spread across
    # the sync/scalar/gpsimd queues in strict priority order ----
    xt = [persist.tile([P, S], BF16, tag=f"xt{k}", name=f"xt{k}")
          for k in range(KC)]
    wv_sb = persist.tile([P, KC, DHC], BF16, tag="wv", name="wv_sb")
    wq_sb = persist.tile([P, KC, DHC], BF16, tag="wq", name="wq_sb")
    wk_sb = persist.tile([P, KC, DHC], BF16, tag="wk", name="wk_sb")
    wo_sb = persist.tile([P, 2, D], BF16, tag="wo", name="wo_sb")

    qs = [nc.sync, nc.scalar, nc.gpsimd]
    qi = [0]

    def dma(out_ap, in_ap):
        qs[qi[0] % 3].dma_start(out=out_ap, in_=in_ap)
        qi[0] += 1

    wv_f = wv_sb[:].rearrange("p k c -> p (k c)")
    wq_f = wq_sb[:].rearrange("p k c -> p (k c)")
    wk_f = wk_sb[:].rearrange("p k c -> p (k c)")
    wo_f = wo_sb[:].rearrange("p k c -> p (k c)")
    # priority set: x j0 + wq + wk first (the first attention group needs
    # scores before values), then wv
    for h in range(4):
        dma(xt[2 * h][:, 0:QG], xT[2 * h * P:(2 * h + 1) * P, 0:QG])
        dma(wq_f[:, h * QG:(h + 1) * QG], wq[:, h * QG:(h + 1) * QG])
        dma(xt[2 * h + 1][:, 0:QG],
            xT[(2 * h + 1) * P:(2 * h + 2) * P, 0:QG])
    for h in range(4):
        dma(wk_f[:, h * QG:(h + 1) * QG], wk[:, h * QG:(h + 1) * QG])
    for h in range(4):
        dma(wv_f[:, h * QG:(h + 1) * QG], wv[:, h * QG:(h + 1) * QG])
    # second wave: x j1, wo
    for k in range(KC):
        dma(xt[k][:, QG:2 * QG], xT[k * P:(k + 1) * P, QG:2 * QG])
    for h in range(4):
        dma(wo_f[:, h * QG:(h + 1) * QG], wo[:, h * QG:(h + 1) * QG])
    # third wave: x j2, j3
    for k in range(KC):
        dma(xt[k][:, 2 * QG:4 * QG], xT[k * P:(k + 1) * P, 2 * QG:4 * QG])

    # ---- PE clock warmup: one accumulation chain so every matmul feeds the
    # observable scrap DRAM write (else DCE drops them) ----
    scrap_d = drampool.tile([1, 16], F32, tag="scrap", name="scrap_d")
    wps = psum.tile([P, QG], F32, tag="pj", name="wps")
    for i in range(16):
        nc.tensor.matmul(wps[:], lhsT=wup[:, 0:P], rhs=wup[:],
                         start=(i == 0), stop=(i == 15))
    scrap_s = consts.tile([1, 16], F32, tag="scrap", name="scrap_s")
    nc.vector.tensor_copy(scrap_s[:], wps[0:1, 0:16])
    nc.gpsimd.dma_start(out=scrap_d[:], in_=scrap_s[:])

    # persistent tensors
    v_sb = [persist.tile([P, HPC, VW], BF16, tag=f"v{s}", name=f"v{s}")
            for s in range(SB)]
    qt = [persist.tile([P, S], BF16, tag=f"qt{i}", name=f"qt{i}")
          for i in range(2)]
    kt = [persist.tile([P, S], BF16, tag=f"kt{i}", name=f"kt{i}")
          for i in range(2)]
    ctxT = [persist.tile([P, S], BF16, tag=f"ctxT{i}", name=f"ctxT{i}")
            for i in range(2)]

    # ---- emission helpers ----
    def emit_v_chunk(sv):
        # two seq blocks (2*sv, 2*sv+1) -> v natural layout
        ps = psum.tile([P, 2, QG], F32, tag="sp", name="vps")
        for par in range(2):
            s = 2 * sv + par
            for k in range(KC):
                nc.tensor.matmul(
                    ps[:, par, 0:DHC],
                    lhsT=xt[k][:, s * P:(s + 1) * P],
                    rhs=wv_sb[:, k, :],
                    start=(k == 0),
                    stop=(k == KC - 1),
                )
            src_ap = ps[:, par, 0:DHC].rearrange("p (h d) -> p h d", h=HPC)
            nc.vector.tensor_copy(v_sb[s][:, :, 0:HD], src_ap)
            nc.vector.memset(v_sb[s][:, :, HD:VW], 1.0)
            nc.vector.memset(v_sb[s][:, :, HD + 1:VW], 0.0)

    def emit_qk_quantum(pair, which, j):
        # one projection accumulation: q or k, d-chunk `pair`, column group j
        w_sb, dst = ((wq_sb, qt) if which == "q" else (wk_sb, kt))
        ps = psum.tile([P, QG], F32, tag="pj", name="pjps")
        for k in range(KC):
            nc.tensor.matmul(
                ps[:],
                lhsT=w_sb[:, k, pair * P:(pair + 1) * P],
                rhs=xt[k][:, j * QG:(j + 1) * QG],
                start=(k == 0),
                stop=(k == KC - 1),
            )
        nc.vector.tensor_copy(dst[pair][:, j * QG:(j + 1) * QG], ps[:])

    out_tiles = {}

    def emit_outproj_half(m, n2, copy_eng=None):
        # out[m*128:(m+1)*128, n2*512:(n2+1)*512] partial projection
        if n2 == 0:
            out_tiles[m] = outpool.tile([P, D], BF16, tag="ot", name="ot")
        ot = out_tiles[m]
        ps = psum.tile([P, QG], F32, tag="pj", name="ops")
        for kc in range(2):
            nc.tensor.matmul(
                ps[:],
                lhsT=ctxT[kc][:, m * P:(m + 1) * P],
                rhs=wo_sb[:, kc, n2 * QG:(n2 + 1) * QG],
                start=(kc == 0),
                stop=(kc == 1),
            )
        if copy_eng == "scalar":
            nc.scalar.copy(ot[:, n2 * QG:(n2 + 1) * QG], ps[:])
        else:
            nc.vector.tensor_copy(ot[:, n2 * QG:(n2 + 1) * QG], ps[:])
        oq = nc.sync if (m + n2) % 2 else nc.gpsimd
        oq.dma_start(out=out[m * P:(m + 1) * P, n2 * QG:(n2 + 1) * QG],
                     in_=ot[:, n2 * QG:(n2 + 1) * QG])
        if n2 == 1:
            del out_tiles[m]

    # ---- filler scheduler: (cost_us, deadline_idx, fn, ready_idx) ----
    fillers = []
    fillers += [(1.7, 1, lambda w=w: emit_qk_quantum(1, w, 0), 0)
                for w in "qk"]
    for g in range(1, 4):
        d0 = 2 * g
        fillers += [(1.7, d0, lambda sv=sv: emit_v_chunk(sv), 0)
                    for sv in (2 * g, 2 * g + 1)]
        fillers += [(1.7, d0, lambda w=w, g=g: emit_qk_quantum(0, w, g), 0)
                    for w in "qk"]
        fillers += [(1.7, min(d0 + 1, 6), lambda w=w, g=g: emit_qk_quantum(1, w, g), 0)
                    for w in "qk"]
    # out-projections for groups 0..1 trail into the ACT-bound later stages;
    # group g's out-proj may only be emitted after norm(1, g) -> ready_idx.
    # Groups 2..3 are reserved for the epilogue (PE filler during the final
    # normalization chain).
    for g in range(2):
        for m in range(4 * g, 4 * g + 4):
            for n2 in range(2):
                fillers.append(
                    (0.5, 99, lambda m=m, n2=n2: emit_outproj_half(m, n2),
                     2 * g + 2))

    state = {"debt": 0.0, "idx": 0}

    def pop_ready(force_deadline=None):
        while fillers:
            c, d, f, r = fillers[0]
            if r > state["idx"]:
                return
            if force_deadline is not None and d <= force_deadline:
                pass  # forced
            elif c > state["debt"]:
                return
            fillers.pop(0)
            f()
            state["debt"] = max(0.0, state["debt"] - c)
            if force_deadline is not None and (not fillers
                                               or fillers[0][1] > force_deadline):
                force_deadline = None

    def emit_attention(pair, g):
        cxs = [psum.tile([VW, QG], F32, tag="cx", name="cx") for _ in range(2)]
        nkb = 4 * g + 4
        es_ts = [None] * nkb
        c0s = [P * (kb - 4 * g) if kb >= 4 * g else 0 for kb in range(nkb)]

        def emit_ctx(kb):
            c0 = c0s[kb]
            for hh in range(2):
                h = 2 * pair + hh
                nc.tensor.matmul(
                    cxs[hh][:, c0:QG],
                    lhsT=v_sb[kb][:, h, :],
                    rhs=es_ts[kb][:, hh, c0:QG],
                    start=(kb == 0),
                    stop=(kb == nkb - 1),
                )
            es_ts[kb] = None

        for kb in range(nkb):
            c0 = c0s[kb]
            w = QG - c0
            sp_t = psum.tile([P, 2, QG], F32, tag="sp", name="sp")
            diag = kb >= 4 * g
            for hh in range(2):
                nc.tensor.matmul(
                    sp_t[:, hh, c0:QG],
                    lhsT=kt[pair][hh * HD:(hh + 1) * HD, kb * P:(kb + 1) * P],
                    rhs=qt[pair][hh * HD:(hh + 1) * HD,
                                 g * QG + c0:(g + 1) * QG],
                    start=True,
                    stop=not diag,
                    skip_group_check=True,
                )
            if diag:
                # accumulate -1e30 on the strict lower local triangle of the
                # diagonal 128-col region, for both heads at once (rhs = I
                # broadcast along the hh axis)
                i_ap = ident[:]
                i_b = bass.AP(i_ap.tensor, i_ap.offset,
                              [i_ap.ap[0], [0, 2], i_ap.ap[1]])
                nc.tensor.matmul(
                    sp_t[:, :, c0:c0 + P],
                    lhsT=maskU[:],
                    rhs=i_b,
                    start=False,
                    stop=True,
                    skip_group_check=True,
                )
            es_t = espool.tile([P, 2, QG], BF16, tag="es", name="es")
            nc.scalar.activation(
                es_t[:, :, c0:QG], sp_t[:, :, c0:QG],
                mybir.ActivationFunctionType.Exp, scale=0.125,
            )
            es_ts[kb] = es_t
            # pipeline: emit ctx for the previous block, fillers in between
            state["debt"] += max(0.0, (2 * w + 420) / 1200.0
                                 - (3 * w / 2400.0 + 0.06))
            if kb >= 1:
                pop_ready()
                emit_ctx(kb - 1)
        pop_ready()
        emit_ctx(nkb - 1)
        return cxs

    def emit_norm(pair, g, cxs):
        # broadcast both heads' rowsum rows (PSUM partition 64) to a full
        # [128, 512] tile via two col-tiled K=1 bf16 matmuls, then one
        # reciprocal and two multiplies.  No DMA involved.
        rbp = psum.tile([P, QG], F32, tag="pj", name="rbp")
        for hh in range(2):
            rs = nrmpool.tile([1, QG], BF16, tag=f"rs{hh}", name="rs")
            nc.vector.tensor_copy(rs[:], cxs[hh][HD:HD + 1, :])
            nc.tensor.matmul(
                rbp[hh * HD:(hh + 1) * HD, :],
                lhsT=ones[:],
                rhs=rs[:],
                start=True,
                stop=True,
                tile_position=(0, hh * HD),
            )
        rbi = nrmpool.tile([P, QG], F32, tag="rbi", name="rbi")
        nc.vector.reciprocal_approx_fast(rbi[:], rbp[:])
        for hh in range(2):
            nc.vector.tensor_mul(
                ctxT[pair][hh * HD:(hh + 1) * HD, g * QG:(g + 1) * QG],
                cxs[hh][0:HD, :],
                rbi[hh * HD:(hh + 1) * HD, :],
            )

    # ---- emission schedule ----
    emit_qk_quantum(0, "q", 0)
    emit_qk_quantum(0, "k", 0)
    emit_v_chunk(0)
    emit_v_chunk(1)
    for idx, (pair, g) in enumerate(ATT_SEQ):
        state["idx"] = idx
        pop_ready(force_deadline=idx)
        cxs = emit_attention(pair, g)
        if idx == len(ATT_SEQ) - 1:
            # drain fillers and the reserved outproj(2) first: they are
            # independent of this group's normalization and keep the PE busy
            # (and the pj ring unblocked) while the norm's DVE chain runs
            state["idx"] = 99
            state["debt"] = 1e9
            pop_ready(force_deadline=99)
            for i, m in enumerate(range(8, 12)):
                for n2 in range(2):
                    emit_outproj_half(m, n2,
                                      copy_eng=("scalar" if (2 * i + n2) % 2
                                                else None))
        emit_norm(pair, g, cxs)
    # epilogue: the final group's out-projection (copies alternate between
    # ScalarE, idle after the last exp, and VectorE so the drain is PE-bound)
    for i, m in enumerate(range(12, 16)):
        for n2 in range(2):
            emit_outproj_half(m, n2,
                              copy_eng=("scalar" if (2 * i + n2) % 2 else None))


def build_nc():
    from contextlib import ExitStack

    nc = bacc.Bacc()
    # weights are host-packed partition-contiguous: w*[p, k*C+c] = W[k*128+p, c]
    io = {
        "xT": nc.dram_tensor("xT", [D, S], BF16, kind="ExternalInput").ap(),
        "wq": nc.dram_tensor("wq", [P, KC * DHC], BF16, kind="ExternalInput").ap(),
        "wk": nc.dram_tensor("wk", [P, KC * DHC], BF16, kind="ExternalInput").ap(),
        "wv": nc.dram_tensor("wv", [P, KC * DHC], BF16, kind="ExternalInput").ap(),
        "wo": nc.dram_tensor("wo", [P, 2 * D], BF16, kind="ExternalInput").ap(),
        "out": nc.dram_tensor("out", [S, D], BF16, kind="ExternalOutput").ap(),
    }
    with tile.TileContext(nc) as tc:
        with ExitStack() as ctx:
            _build_body(ctx, tc, io)
    nc.finalize()
    return nc


_NC = None


def _get_nc():
    global _NC
    if _NC is None:
        _NC = build_nc()
    return _NC


def _pack(w, nchunk):
    # [nchunk*128, C] -> [128, nchunk*C] with w_packed[p, k*C+c] = w[k*128+p, c]
    bf = ml_dtypes.bfloat16
    w = np.asarray(w, np.float32)
    C = w.shape[1]
    return np.ascontiguousarray(
        w.reshape(nchunk, P, C).transpose(1, 0, 2).reshape(P, nchunk * C)
    ).astype(bf)


def make_in_maps(x, Wq, Wk, Wv, Wo):
    bf = ml_dtypes.bfloat16
    x = np.asarray(x, dtype=np.float32)
    Wq, Wk, Wv = (np.asarray(w, np.float32) for w in (Wq, Wk, Wv))
    Wo = np.asarray(Wo, np.float32)
    in_maps = []
    xTs = [np.ascontiguousarray(x[b].T).astype(bf) for b in range(B)]
    for c in range(NCORES):
        b, g = divmod(c, 4)
        sl = slice(DHC * g, DHC * (g + 1))
        in_maps.append({
            "xT": xTs[b],
            "wq": _pack(Wq[:, sl], KC),
            "wk": _pack(Wk[:, sl], KC),
            "wv": _pack(Wv[:, sl], KC),
            "wo": _pack(Wo[sl, :], 2),
        })
    return in_maps


def run(in_maps, trace=False, **kw):
    return run_bass_kernel_spmd(_get_nc(), in_maps, list(range(NCORES)),
                                trace=trace, **kw)


def kernel(x, Wq, Wk, Wv, Wo, bo):
    res = run(make_in_maps(x, Wq, Wk, Wv, Wo)).results
    bo = np.asarray(bo, np.float32)
    out = np.empty((B, S, D), np.float32)
    for b in range(B):
        acc = res[4 * b]["out"].astype(np.float32)
        for g in range(1, 4):
            acc = acc + res[4 * b + g]["out"].astype(np.float32)
        out[b] = acc + bo[None, :]
    return out
